# revision 19
# baseline (speedup 1.0000x reference)
"""ChrEmbed (per-chromosome Dense stack) Trainium2 kernel.

Computes out[b, c, :] = x[:, off_c:off_c+n_c] @ Ws[c] + bs[c] for the 22
chromosome blocks, stacked to [512, 22, 256].

Strategy: 8-way expert parallelism with a uniform SPMD program (one NEFF on
all 8 NeuronCores, per-core data).
  - Host transposes x during sharding (contraction must sit on the SBUF
    partition axis; fp32 DMA-transpose doesn't exist, numpy does it free).
  - The ragged chromosomes (11..63 blocks of 128 features) are chopped into
    chunks packed into 8 cores x 8 slots with per-slot capacities
    PROFILE = [16,16,16,14,13,9,7,4] blocks (95 blocks/core, 1.9% padding).
    Every core runs the same instruction stream on its own chunk data.
  - Per slot: stream xT and W tiles (x on the SyncE HWDGE ring, W on the
    ScalarE ring); per contraction block kb: matmuls with W[128,128] halves
    STATIONARY and xT [128, 512] MOVING (N=512 amortizes weight loads),
    accumulating into 2 PSUM banks [128u, 512b]; DVE-evacuate (cast to the
    output dtype), DMA out.
  - Outputs land u-major [u, b]; the host un-transposes during gather and
    sums partial chunks of the same chromosome, then adds biases.

Measured on trn2 (8 axon NeuronCores): f16 scheme ~70-80us NEFF exec
(pair-shared HBM roofline: 2x20.9MB / 716GB/s + ~9us preamble), max
scale-relative err 4.4e-4, resid_var 1.3e-7 vs the fp32 reference.

SCHEME selects the matmul precision/byte tradeoff (host casts during shard;
all measured on HW in this structure):
  f32   : exact fp32 matmul (4 cyc/row, PE-bound): 186us, rel err 7.2e-7
  f32r  : fp32 storage, TF32-class multiplies:     115us, rel err 1.5e-4
  f16x3 : x and W split into fp16 hi+lo, 3 cross terms (~1e-6 err, PE-bound)
  f16x2 : x split fp16 hi+lo, W single fp16 (~2.2e-4 err, 0.75x bytes)
  f16   : everything single fp16 (default):      68-78us, rel err 4.4e-4,
          resid_var 1.3e-7 -- 1000x inside the 1e-4 resid_var kernel-test
          convention, and the fastest: the kernel sits on the HBM roofline
          (two cores share a 716 GB/s stack; 2 x 20.9MB -> ~59us + 9us
          fixed NEFF preamble).
"""

import os

import numpy as np

import concourse.mybir as mybir
import concourse.tile as tile
from concourse import bacc
from concourse.bass_utils import run_bass_kernel_spmd

# ---- problem constants (hardcoded; kernel.py must be self-contained) ----
SNP2CHR = [8000, 7800, 6600, 6200, 6000, 5800, 5200, 5000, 4400, 4600, 4500,
           4400, 3400, 3200, 3000, 3000, 2800, 2700, 2200, 2200, 1400, 1600]
OFFSETS = np.concatenate([[0], np.cumsum(SNP2CHR)]).astype(np.int64)
N_CHR = len(SNP2CHR)
UNITS = 256
BATCH = 512
N_CORES = 8

P = 128                                  # partition / contraction block
PROFILE = [16, 16, 16, 14, 13, 9, 7, 4]  # per-core slot capacities (blocks)
S = len(PROFILE)
C_BLK = sum(PROFILE)                     # 95 blocks per core
SLOT_OFF = np.concatenate([[0], np.cumsum(PROFILE)]).astype(int)

SCHEME = os.environ.get("CHREMBED_SCHEME", "f16")

_CFG = {
    #        nx  nw  dtype
    "f32":  (1, 1, mybir.dt.float32),
    "f32r": (1, 1, mybir.dt.float32r),
    "f16":  (1, 1, mybir.dt.float16),
    "f16x2": (2, 1, mybir.dt.float16),
    "f16x3": (2, 2, mybir.dt.float16),
}


def _pack_chunks():
    """Chop chromosomes into chunks fitting the slot-size pool (8 copies of
    PROFILE) and assign each chunk to a (core, slot). Greedy best-fit.
    Returns list of (chrom, col_start, ncols, core, slot)."""
    from collections import Counter

    pool = Counter()
    for L in PROFILE:
        pool[L] += N_CORES
    slot_ids = {}
    for sz in set(PROFILE):
        ids = []
        for core in range(N_CORES):
            for si, L in enumerate(PROFILE):
                if L == sz:
                    ids.append((core, si))
        slot_ids[sz] = ids

    order = sorted(range(N_CHR), key=lambda c: -SNP2CHR[c])
    used = Counter()
    chunks = []
    for c in order:
        rem_rows = SNP2CHR[c]
        col = int(OFFSETS[c])
        while rem_rows > 0:
            rem_blk = -(-rem_rows // P)
            sizes = sorted((s for s in pool if pool[s] > 0), reverse=True)
            assert sizes, "profile infeasible"
            if rem_blk >= sizes[0]:
                take = sizes[0]
            else:
                cands = [s for s in sizes if s >= rem_blk]
                take = min(cands) if cands else sizes[0]
            pool[take] -= 1
            core, si = slot_ids[take][used[take]]
            used[take] += 1
            ncols = min(take * P, rem_rows)
            chunks.append((c, col, ncols, core, si))
            col += ncols
            rem_rows -= ncols
    return chunks


CHUNKS = _pack_chunks()

_NC_CACHE = {}


def _build_program(scheme):
    if scheme in _NC_CACHE:
        return _NC_CACHE[scheme]
    nx, nw, dt_mm = _CFG[scheme]
    f32 = mybir.dt.float32
    dt_out = mybir.dt.float16 if scheme == "f16" else f32
    XW = nx * BATCH      # x elements per block (all terms)
    WW = nw * UNITS      # w elements per block

    nc = bacc.Bacc("TRN2", target_bir_lowering=False, debug=False)
    xT_d = nc.dram_tensor("xT", (P, C_BLK * XW), dt_mm, kind="ExternalInput")
    w_d = nc.dram_tensor("w", (P, C_BLK * WW), dt_mm, kind="ExternalInput")
    out_d = nc.dram_tensor("out", (S, P, 2 * BATCH), dt_out, kind="ExternalOutput")

    with tile.TileContext(nc) as tc:
        with (
            tc.tile_pool(name="xp", bufs=3) as xp,
            tc.tile_pool(name="wp", bufs=3) as wp,
            tc.tile_pool(name="op", bufs=3) as op,
            tc.tile_pool(name="ps", bufs=3, space="PSUM") as ps,
        ):
            for s in range(S):
                L = PROFILE[s]
                off = int(SLOT_OFF[s])
                xt = xp.tile([P, L * XW], dt_mm, tag="x", name=f"xt{s}")
                wt = wp.tile([P, L * WW], dt_mm, tag="w", name=f"wt{s}")
                # First slots: split DMAs so the first matmuls start early.
                nsplit = 4 if s == 0 else (2 if s == 1 else 1)
                bnds = [L * i // nsplit for i in range(nsplit + 1)]
                for a, b in zip(bnds[:-1], bnds[1:]):
                    nc.sync.dma_start(
                        xt[:, a * XW:b * XW], xT_d[:, (off + a) * XW:(off + b) * XW]
                    )
                    nc.scalar.dma_start(
                        wt[:, a * WW:b * WW], w_d[:, (off + a) * WW:(off + b) * WW]
                    )
                psums = [
                    ps.tile([P, BATCH], f32, tag=f"ps{h}", name=f"psum{s}_{h}")
                    for h in range(2)
                ]
                # terms: (wi, xi) pairs; skip lo*lo for f16x3
                terms = [(wi, xi) for wi in range(nw) for xi in range(nx)
                         if wi + xi < max(nx, nw)]
                for kb in range(L):
                    first, last = (kb == 0), (kb == L - 1)
                    for h in range(2):
                        for ti, (wi, xi) in enumerate(terms):
                            lhsT = wt[:, kb * WW + wi * UNITS + h * P:
                                      kb * WW + wi * UNITS + (h + 1) * P]
                            rhs = xt[:, kb * XW + xi * BATCH:
                                     kb * XW + (xi + 1) * BATCH]
                            nc.tensor.matmul(
                                psums[h][:], lhsT, rhs,
                                start=first and ti == 0,
                                stop=last and ti == len(terms) - 1,
                            )
                ot = op.tile([P, 2 * BATCH], dt_out, tag="o", name=f"ot{s}")
                for h in range(2):
                    nc.vector.tensor_copy(
                        ot[:, h * BATCH:(h + 1) * BATCH], psums[h][:]
                    )
                nc.scalar.dma_start(out_d[s], ot[:])

    nc.compile()
    _NC_CACHE[scheme] = nc
    return nc


def _split_terms(a, n_terms, np_dt):
    """Represent fp32 array as sum of n_terms arrays of dtype np_dt."""
    if n_terms == 1:
        return [np.ascontiguousarray(a, np_dt)]
    hi = a.astype(np_dt)
    lo = (a - hi.astype(np.float32)).astype(np_dt)
    return [hi, lo]


def _shard_inputs(x, Ws, scheme):
    nx, nw, dt_mm = _CFG[scheme]
    np_dt = mybir.dt.np(dt_mm)
    XW = nx * BATCH
    WW = nw * UNITS
    in_maps = [
        {
            "xT": np.zeros((P, C_BLK * XW), np_dt),
            "w": np.zeros((P, C_BLK * WW), np_dt),
        }
        for _ in range(N_CORES)
    ]
    for (c, col0, ncols, core, si) in CHUNKS:
        L = PROFILE[si]
        off = int(SLOT_OFF[si])
        rel0 = col0 - int(OFFSETS[c])

        xterms = _split_terms(x[:, col0:col0 + ncols].T, nx, np_dt)
        xbuf = np.zeros((L * P, nx, BATCH), np_dt)
        for xi, t in enumerate(xterms):
            xbuf[:ncols, xi, :] = t
        in_maps[core]["xT"][:, off * XW:(off + L) * XW] = (
            xbuf.reshape(L, P, XW).swapaxes(0, 1).reshape(P, L * XW)
        )

        wterms = _split_terms(Ws[c][rel0:rel0 + ncols], nw, np_dt)
        wbuf = np.zeros((L * P, nw, UNITS), np_dt)
        for wi, t in enumerate(wterms):
            wbuf[:ncols, wi, :] = t
        in_maps[core]["w"][:, off * WW:(off + L) * WW] = (
            wbuf.reshape(L, P, WW).swapaxes(0, 1).reshape(P, L * WW)
        )
    return in_maps


def _gather(results, bs):
    out = np.zeros((BATCH, N_CHR, UNITS), np.float32)
    for (c, _col0, _ncols, core, si) in CHUNKS:
        r = results[core]["out"][si]                       # [P, 2*BATCH]
        part = r.reshape(P, 2, BATCH).transpose(1, 0, 2)   # [2, P(u), b]
        out[:, c, :] += part.reshape(2 * P, BATCH).T       # [b, u]
    for c in range(N_CHR):
        b = np.asarray(bs[c], np.float32)
        if b.any():
            out[:, c, :] += b
    return out


def kernel(x, Ws, bs, _run_kwargs=None):
    x = np.asarray(x, np.float32)
    Ws = [np.asarray(w, np.float32) for w in Ws]
    nc = _build_program(SCHEME)
    in_maps = _shard_inputs(x, Ws, SCHEME)
    res = run_bass_kernel_spmd(
        nc, in_maps, core_ids=list(range(N_CORES)), **(_run_kwargs or {})
    )
    out = _gather(res.results, bs)
    if _run_kwargs:
        kernel.last_result = res
    return out


# revision 21
# speedup vs baseline: 1.2094x; 1.2094x over previous
"""ChrEmbed (per-chromosome Dense stack) Trainium2 kernel.

Computes out[b, c, :] = x[:, off_c:off_c+n_c] @ Ws[c] + bs[c] for the 22
chromosome blocks, stacked to [512, 22, 256].

Strategy: 8-way expert parallelism with a uniform SPMD program (one NEFF on
all 8 NeuronCores, per-core data).
  - Host transposes x during sharding (contraction must sit on the SBUF
    partition axis; fp32 DMA-transpose doesn't exist, numpy does it free).
  - The ragged chromosomes (11..63 blocks of 128 features) are chopped into
    chunks packed into 8 cores x 8 slots with per-slot capacities
    PROFILE = [16,16,16,14,13,9,7,4] blocks (95 blocks/core, 1.9% padding).
    Every core runs the same instruction stream on its own chunk data.
  - Per slot: stream xT and W tiles (x on the SyncE HWDGE ring, W on the
    ScalarE ring); per contraction block kb: matmuls with W[128,128] halves
    STATIONARY and xT [128, 512] MOVING (N=512 amortizes weight loads),
    accumulating into 2 PSUM banks [128u, 512b]; DVE-evacuate (cast to the
    output dtype), DMA out.
  - Outputs land u-major [u, b]; the host un-transposes during gather and
    sums partial chunks of the same chromosome, then adds biases.

Measured on trn2 (8 axon NeuronCores): f16 scheme ~70-80us NEFF exec
(pair-shared HBM roofline: 2x20.9MB / 716GB/s + ~9us preamble), max
scale-relative err 4.4e-4, resid_var 1.3e-7 vs the fp32 reference.

SCHEME selects the matmul precision/byte tradeoff (host casts during shard;
all measured on HW in this structure):
  f32   : exact fp32 matmul (4 cyc/row, PE-bound): 186us, rel err 7.2e-7
  f32r  : fp32 storage, TF32-class multiplies:     115us, rel err 1.5e-4
  f16x3 : x and W split into fp16 hi+lo, 3 cross terms (~1e-6 err, PE-bound)
  f16x2 : x split fp16 hi+lo, W single fp16 (~2.2e-4 err, 0.75x bytes)
  f16   : everything single fp16 (default):      68-78us, rel err 4.4e-4,
          resid_var 1.3e-7 -- 1000x inside the 1e-4 resid_var kernel-test
          convention, and the fastest: the kernel sits on the HBM roofline
          (two cores share a 716 GB/s stack; 2 x 20.9MB -> ~59us + 9us
          fixed NEFF preamble).
"""

import os

import numpy as np

import concourse.mybir as mybir
import concourse.tile as tile
from concourse import bacc
from concourse.bass_utils import run_bass_kernel_spmd

# ---- problem constants (hardcoded; kernel.py must be self-contained) ----
SNP2CHR = [8000, 7800, 6600, 6200, 6000, 5800, 5200, 5000, 4400, 4600, 4500,
           4400, 3400, 3200, 3000, 3000, 2800, 2700, 2200, 2200, 1400, 1600]
OFFSETS = np.concatenate([[0], np.cumsum(SNP2CHR)]).astype(np.int64)
N_CHR = len(SNP2CHR)
UNITS = 256
BATCH = 512
N_CORES = 8

P = 128                                  # partition / contraction block
PROFILE = [16, 16, 16, 14, 13, 9, 7, 4]  # per-core slot capacities (blocks)
S = len(PROFILE)
C_BLK = sum(PROFILE)                     # 95 blocks per core
SLOT_OFF = np.concatenate([[0], np.cumsum(PROFILE)]).astype(int)

SCHEME = os.environ.get("CHREMBED_SCHEME", "f16")

_CFG = {
    #        nx  nw  dtype
    "f32":  (1, 1, mybir.dt.float32),
    "f32r": (1, 1, mybir.dt.float32r),
    "f16":  (1, 1, mybir.dt.float16),
    "f16x2": (2, 1, mybir.dt.float16),
    "f16x3": (2, 2, mybir.dt.float16),
}


def _pack_chunks():
    """Chop chromosomes into chunks fitting the slot-size pool (8 copies of
    PROFILE) and assign each chunk to a (core, slot). Greedy best-fit.
    Returns list of (chrom, col_start, ncols, core, slot)."""
    from collections import Counter

    pool = Counter()
    for L in PROFILE:
        pool[L] += N_CORES
    slot_ids = {}
    for sz in set(PROFILE):
        ids = []
        for core in range(N_CORES):
            for si, L in enumerate(PROFILE):
                if L == sz:
                    ids.append((core, si))
        slot_ids[sz] = ids

    order = sorted(range(N_CHR), key=lambda c: -SNP2CHR[c])
    used = Counter()
    chunks = []
    for c in order:
        rem_rows = SNP2CHR[c]
        col = int(OFFSETS[c])
        while rem_rows > 0:
            rem_blk = -(-rem_rows // P)
            sizes = sorted((s for s in pool if pool[s] > 0), reverse=True)
            assert sizes, "profile infeasible"
            if rem_blk >= sizes[0]:
                take = sizes[0]
            else:
                cands = [s for s in sizes if s >= rem_blk]
                take = min(cands) if cands else sizes[0]
            pool[take] -= 1
            core, si = slot_ids[take][used[take]]
            used[take] += 1
            ncols = min(take * P, rem_rows)
            chunks.append((c, col, ncols, core, si))
            col += ncols
            rem_rows -= ncols
    return chunks


CHUNKS = _pack_chunks()

_NC_CACHE = {}


def _build_program(scheme):
    if scheme in _NC_CACHE:
        return _NC_CACHE[scheme]
    nx, nw, dt_mm = _CFG[scheme]
    f32 = mybir.dt.float32
    dt_out = mybir.dt.float16 if scheme == "f16" else f32
    XW = nx * BATCH      # x elements per block (all terms)
    WW = nw * UNITS      # w elements per block

    nc = bacc.Bacc("TRN2", target_bir_lowering=False, debug=False)
    xT_d = nc.dram_tensor("xT", (P, C_BLK * XW), dt_mm, kind="ExternalInput")
    w_d = nc.dram_tensor("w", (P, C_BLK * WW), dt_mm, kind="ExternalInput")
    out_d = nc.dram_tensor("out", (S, P, 2 * BATCH), dt_out, kind="ExternalOutput")

    with tile.TileContext(nc) as tc:
        with (
            tc.tile_pool(name="xp", bufs=3) as xp,
            tc.tile_pool(name="wp", bufs=3) as wp,
            tc.tile_pool(name="op", bufs=2) as op,
            tc.tile_pool(name="ps", bufs=3, space="PSUM") as ps,
        ):
            for s in range(S):
                L = PROFILE[s]
                off = int(SLOT_OFF[s])
                xt = xp.tile([P, L * XW], dt_mm, tag="x", name=f"xt{s}")
                wt = wp.tile([P, L * WW], dt_mm, tag="w", name=f"wt{s}")
                # First slots: split DMAs so the first matmuls start early.
                nsplit = 4 if s == 0 else (2 if s == 1 else 1)
                bnds = [L * i // nsplit for i in range(nsplit + 1)]
                for a, b in zip(bnds[:-1], bnds[1:]):
                    nc.sync.dma_start(
                        xt[:, a * XW:b * XW], xT_d[:, (off + a) * XW:(off + b) * XW]
                    )
                    nc.scalar.dma_start(
                        wt[:, a * WW:b * WW], w_d[:, (off + a) * WW:(off + b) * WW]
                    )
                psums = [
                    ps.tile([P, BATCH], f32, tag=f"ps{h}", name=f"psum{s}_{h}")
                    for h in range(2)
                ]
                # terms: (wi, xi) pairs; skip lo*lo for f16x3
                terms = [(wi, xi) for wi in range(nw) for xi in range(nx)
                         if wi + xi < max(nx, nw)]
                for kb in range(L):
                    first, last = (kb == 0), (kb == L - 1)
                    for h in range(2):
                        for ti, (wi, xi) in enumerate(terms):
                            lhsT = wt[:, kb * WW + wi * UNITS + h * P:
                                      kb * WW + wi * UNITS + (h + 1) * P]
                            rhs = xt[:, kb * XW + xi * BATCH:
                                     kb * XW + (xi + 1) * BATCH]
                            nc.tensor.matmul(
                                psums[h][:], lhsT, rhs,
                                start=first and ti == 0,
                                stop=last and ti == len(terms) - 1,
                            )
                ot = op.tile([P, 2 * BATCH], dt_out, tag="o", name=f"ot{s}")
                for h in range(2):
                    nc.vector.tensor_copy(
                        ot[:, h * BATCH:(h + 1) * BATCH], psums[h][:]
                    )
                    nc.scalar.dma_start(
                        out_d[s, :, h * BATCH:(h + 1) * BATCH],
                        ot[:, h * BATCH:(h + 1) * BATCH],
                    )

    nc.compile()
    _NC_CACHE[scheme] = nc
    return nc


def _split_terms(a, n_terms, np_dt):
    """Represent fp32 array as sum of n_terms arrays of dtype np_dt."""
    if n_terms == 1:
        return [np.ascontiguousarray(a, np_dt)]
    hi = a.astype(np_dt)
    lo = (a - hi.astype(np.float32)).astype(np_dt)
    return [hi, lo]


def _shard_inputs(x, Ws, scheme):
    nx, nw, dt_mm = _CFG[scheme]
    np_dt = mybir.dt.np(dt_mm)
    XW = nx * BATCH
    WW = nw * UNITS
    in_maps = [
        {
            "xT": np.zeros((P, C_BLK * XW), np_dt),
            "w": np.zeros((P, C_BLK * WW), np_dt),
        }
        for _ in range(N_CORES)
    ]
    for (c, col0, ncols, core, si) in CHUNKS:
        L = PROFILE[si]
        off = int(SLOT_OFF[si])
        rel0 = col0 - int(OFFSETS[c])

        xterms = _split_terms(x[:, col0:col0 + ncols].T, nx, np_dt)
        xbuf = np.zeros((L * P, nx, BATCH), np_dt)
        for xi, t in enumerate(xterms):
            xbuf[:ncols, xi, :] = t
        in_maps[core]["xT"][:, off * XW:(off + L) * XW] = (
            xbuf.reshape(L, P, XW).swapaxes(0, 1).reshape(P, L * XW)
        )

        wterms = _split_terms(Ws[c][rel0:rel0 + ncols], nw, np_dt)
        wbuf = np.zeros((L * P, nw, UNITS), np_dt)
        for wi, t in enumerate(wterms):
            wbuf[:ncols, wi, :] = t
        in_maps[core]["w"][:, off * WW:(off + L) * WW] = (
            wbuf.reshape(L, P, WW).swapaxes(0, 1).reshape(P, L * WW)
        )
    return in_maps


def _gather(results, bs):
    out = np.zeros((BATCH, N_CHR, UNITS), np.float32)
    for (c, _col0, _ncols, core, si) in CHUNKS:
        r = results[core]["out"][si]                       # [P, 2*BATCH]
        part = r.reshape(P, 2, BATCH).transpose(1, 0, 2)   # [2, P(u), b]
        out[:, c, :] += part.reshape(2 * P, BATCH).T       # [b, u]
    for c in range(N_CHR):
        b = np.asarray(bs[c], np.float32)
        if b.any():
            out[:, c, :] += b
    return out


def kernel(x, Ws, bs, _run_kwargs=None):
    x = np.asarray(x, np.float32)
    Ws = [np.asarray(w, np.float32) for w in Ws]
    nc = _build_program(SCHEME)
    in_maps = _shard_inputs(x, Ws, SCHEME)
    res = run_bass_kernel_spmd(
        nc, in_maps, core_ids=list(range(N_CORES)), **(_run_kwargs or {})
    )
    out = _gather(res.results, bs)
    if _run_kwargs:
        kernel.last_result = res
    return out
